# revision 63
# baseline (speedup 1.0000x reference)
"""Single-head causal attention (B=8, T=2048, C=1024, head_dim=64) on 8 TRN2 NeuronCores.

Sharding: data-parallel over batch -- one batch element per core, qkv weights
replicated. Host prep per core: x[b] transposed to [C, T] fp16; W pre-packed into
the SBUF chunk layout.

v7 schedule. Tile dependencies are tile-granular, so everything the attention
phase consumes is split into PER-GROUP tiles (kTg/qTg/vTg/v1g per 512-column
group) and ST matmuls are split at group boundaries; attention group g then
only waits on group <= g evacuations, not the whole projection epilogue.
  - projection: kqT/vT accumulate j-outer in 8 PSUM banks, paced by chunked
    x DMAs (first chunk split in four so the first matmul starts early).
  - boundary: K/Q/V evacuated per group with fused bias+cast (Q straight
    from PSUM rows 64:128 to SBUF rows 0:64 -- engines can shift partition
    base); Vector does groups 0-1, Scalar groups 2-3; v1 tiles via
    DMA-transpose XBAR on sync. PE warmups (targeting the first-freed PSUM
    bank) bridge the pipeline-fill latency to hold the 2.4GHz p-state.
  - attention (t-group outer): ST pieces -> exp on Scalar -> diag masks on
    Vector -> PV accumulate [65, 512] (denominator in row 64 via the ones
    column of v1). Output leaves UNNORMALIZED as [4, 65, 512] f32 straight
    from an SBUF copy; host divides + transposes.
"""

import numpy as np

import concourse.bass as bass
import concourse.mybir as mybir
from concourse import bacc
from concourse.bass import ts
from concourse.bass_utils import run_bass_kernel_spmd
from concourse.tile import TileContext

B, T, C = 8, 2048, 1024
HD = 64
N_CORES = 8
NJ = C // 128  # contraction chunks for the qkv projection
NT = T // 128  # 128-row tiles along T
NG = T // 512  # 512-col groups along T
FP16 = mybir.dt.float16
IDENT = mybir.ActivationFunctionType.Identity
CST_W = 8 * 192 + 2 + 128 + 64 + 128 + 128  # 1986 (host layout kept from v1)
CB = 192  # cstA holds cols [0:192) = W chunk 0; cstB the rest
F32 = mybir.dt.float32
EXP = mybir.ActivationFunctionType.Exp


def build_nc() -> bass.Bass:
    nc = bacc.Bacc(None, target_bir_lowering=False)
    # w is pre-packed on host: [128, NJ*192] with w[p, j*192+m] = W[j*128+p, m]
    xt = nc.declare_dram_parameter("xt", [C, T], FP16, isOutput=False)
    cst = nc.declare_dram_parameter("cst", [128, CST_W], FP16, isOutput=False)
    # unnormalized output: per 512-col group, rows 0:64 = sum(P v), row 64 = sum(P)
    out = nc.declare_dram_parameter("out", [NG, HD + 1, 512], F32, isOutput=True)

    with TileContext(nc) as tc:
        with (
            tc.tile_pool(name="consts", bufs=1) as consts,
            tc.tile_pool(name="xtp", bufs=NJ + 3) as xtp,
            tc.tile_pool(name="kqv", bufs=1) as kqv,
            tc.tile_pool(name="ptp", bufs=4) as ptp,
            tc.tile_pool(name="osb", bufs=2) as osb,
        ):
            # per-chunk W tiles (separate tiles => per-chunk DMA dependencies)
            cstW = [consts.tile([128, CB], FP16, name=f"cstW{j}") for j in range(NJ)]
            cstM = consts.tile([128, CST_W - 1536], FP16)  # biases | msk
            msk_sb = cstM[:, 2:130]
            wu_sb = consts.tile([1, 512], FP16)
            nc.vector.memset(wu_sb[:], 1.0)
            bias32 = consts.tile([128, 2], F32)

            kTg = [kqv.tile([64, 512], FP16, name=f"kTg{n}") for n in range(NG)]
            qTg = [kqv.tile([64, 512], FP16, name=f"qTg{n}") for n in range(NG)]
            # v for group pairs (0,1) and (2,3) stacked on the partition axis
            vTp = [kqv.tile([128, 512], FP16, name=f"vTp{h}") for h in range(2)]
            v1g = [kqv.tile([128, 4, 80], FP16, name=f"v1g{n}") for n in range(NG)]
            for n in range(NG):
                nc.vector.memset(v1g[n][:, :, HD:HD + 1], 1.0)

            # --- DMAs: interleaved so W chunk j and x chunk j both land just
            # before the projection needs them (per-queue FIFO, ~150 GB/s each)
            xts = [xtp.tile([128, T], FP16, tag="xt", name=f"xt{j}") for j in range(NJ)]
            xt0h = [xtp.tile([128, 1024], FP16, tag="xt0", name=f"xt0h{h}") for h in range(2)]
            with tc.high_priority():
                nc.scalar.dma_start(out=cstW[0][:], in_=cst[:, 0:CB])
                nc.sync.dma_start(out=xt0h[0][:], in_=xt[0:128, 0:1024])
            nc.sync.dma_start(out=xt0h[1][:], in_=xt[0:128, 1024:2048])
            nc.scalar.dma_start(out=cstW[1][:], in_=cst[:, CB:2 * CB])
            nc.sync.dma_start(out=xts[1][:], in_=xt[128:256, :])
            nc.scalar.dma_start(out=xts[2][:], in_=xt[256:384, :])
            nc.scalar.dma_start(out=cstW[2][:], in_=cst[:, 2 * CB:3 * CB])
            nc.scalar.dma_start(out=cstW[3][:], in_=cst[:, 3 * CB:4 * CB])
            nc.scalar.dma_start(out=xts[3][:], in_=xt[384:512, :])
            nc.sync.dma_start(out=xts[4][:], in_=xt[512:640, :])
            nc.scalar.dma_start(out=cstW[4][:], in_=cst[:, 4 * CB:5 * CB])
            nc.scalar.dma_start(out=cstW[5][:], in_=cst[:, 5 * CB:6 * CB])
            nc.scalar.dma_start(out=xts[5][:], in_=xt[640:768, :])
            nc.sync.dma_start(out=xts[6][:], in_=xt[768:896, :])
            nc.scalar.dma_start(out=cstW[6][:], in_=cst[:, 6 * CB:7 * CB])
            nc.scalar.dma_start(out=cstW[7][:], in_=cst[:, 7 * CB:8 * CB])
            nc.scalar.dma_start(out=xts[7][:], in_=xt[896:1024, :])
            nc.sync.dma_start(out=cstM[:], in_=cst[:, 1536:])
            nc.vector.tensor_copy(bias32[:, 0:1], cstM[:, 0:1])
            nc.vector.tensor_copy(bias32[0:64, 1:2], cstM[0:64, 1:2])

            with tc.tile_pool(name="psp", bufs=6, space=bass.MemorySpace.PSUM) as psp:
                # PSUM bank choreography (empirical: the attention pst pool's
                # first slot lands on banks 4-5 = psp emission tiles #5,#6).
                # v accumulators for group pairs share banks: group n at
                # partitions 0:64, group n+1 at 64:128. 6 banks total, leaving
                # two banks of slack for the attention pools.
                kq_accs, vsh = [None] * NG, [None] * 2
                kq_accs[1] = psp.tile([128, 512], F32, tag="p", name="kq_acc1")
                kq_accs[2] = psp.tile([128, 512], F32, tag="p", name="kq_acc2")
                vsh[1] = psp.tile([128, 512], F32, tag="p", name="vsh1")
                kq_accs[3] = psp.tile([128, 512], F32, tag="p", name="kq_acc3")
                kq_accs[0] = psp.tile([128, 512], F32, tag="p", name="kq_acc0")
                vsh[0] = psp.tile([128, 512], F32, tag="p", name="vsh0")
                v_accs = [vsh[0][0:64, :], vsh[0][64:128, :],
                          vsh[1][0:64, :], vsh[1][64:128, :]]
                # no lead warmups: the boost budget is a power duty-cycle --
                # every wasted column costs; the ramp clock starts when the
                # first real matmul issues
                for j in range(NJ):
                    first, last = j == 0, j == NJ - 1
                    if last:  # interleave so accumulators stop (and free) early
                        order = [kv for n in range(NG) for kv in (("kq", n), ("v", n))]
                    else:
                        order = [("kq", n) for n in range(NG)] + [("v", n) for n in range(NG)]
                    for kind, n in order:
                        if j == 0:
                            src = xt0h[n // 2][:, ts(n % 2, 512)]
                        else:
                            src = xts[j][:, ts(n, 512)]
                        if kind == "kq":
                            nc.tensor.matmul(
                                kq_accs[n][:], cstW[j][:, 0:128], src,
                                start=first, stop=last,
                            )
                        else:
                            nc.tensor.matmul(
                                v_accs[n], cstW[j][:, 128:192], src,
                                start=first, stop=last, skip_group_check=True,
                            )
                # boundary: evacuate K/Q per group with fused bias + cast (Q
                # straight from PSUM rows 64:128 to SBUF rows 0:64); the V
                # bias is separable through the softmax and added on HOST, so
                # each V group-pair evacuates in ONE fused cast-copy.
                # Vector: groups 0-1 + V01; Scalar: groups 2-3 + V23.
                ev_dve = lambda o, i, b: nc.vector.tensor_scalar_add(o, i, b)
                ev_act = lambda o, i, b: nc.scalar.activation(o, i, IDENT, bias=b)
                # DVE chain: group 0 first, then V pair 01, group 1, group 3
                ev_dve(kTg[0][:], kq_accs[0][0:64, :], bias32[0:64, 0:1])
                ev_dve(qTg[0][:], kq_accs[0][64:128, :], bias32[64:128, 0:1])
                nc.vector.tensor_copy(vTp[0][:], vsh[0][:])
                nc.sync.dma_start(
                    out=v1g[0][:, 0:2, 0:HD], in_=vTp[0][0:64, 0:256], transpose=True)
                nc.sync.dma_start(
                    out=v1g[0][:, 2:4, 0:HD], in_=vTp[0][0:64, 256:512], transpose=True)
                ev_dve(kTg[1][:], kq_accs[1][0:64, :], bias32[0:64, 0:1])
                ev_dve(qTg[1][:], kq_accs[1][64:128, :], bias32[64:128, 0:1])
                # ACT chain
                ev_act(kTg[2][:], kq_accs[2][0:64, :], bias32[0:64, 0:1])
                ev_act(qTg[2][:], kq_accs[2][64:128, :], bias32[64:128, 0:1])
                nc.scalar.copy(vTp[1][:], vsh[1][:])
                nc.sync.dma_start(
                    out=v1g[2][:, :, 0:HD], in_=vTp[1][0:64, :], transpose=True)
                nc.sync.dma_start(
                    out=v1g[3][:, :, 0:HD], in_=vTp[1][64:128, :], transpose=True)
                ev_act(kTg[3][:], kq_accs[3][0:64, :], bias32[0:64, 0:1])
                ev_act(qTg[3][:], kq_accs[3][64:128, :], bias32[64:128, 0:1])
                nc.sync.dma_start(
                    out=v1g[1][:, :, 0:HD], in_=vTp[0][64:128, :], transpose=True)
                # minimal boundary filler: enough to cover the evac/exp/v1
                # pipeline-fill, small enough not to waste the boost window
                for r in range(2):
                    nc.tensor.matmul(kq_accs[0][:], wu_sb[:, 0:128], wu_sb[:], start=True, stop=True)

            # --- attention, t-group outer: ST pieces for two s-chunks share a
            # [128,1024] PSUM tile and one exp (Scalar does ONLY exp); diag
            # masks on Vector; PV accumulates [65, 512], denominator in row 64 ---
            with (
                tc.tile_pool(name="pst", bufs=3, space=bass.MemorySpace.PSUM) as pst,
                tc.tile_pool(name="pso", bufs=2, space=bass.MemorySpace.PSUM) as pso,
            ):
                def do_pair(g, p, acc, jstop=None):
                    gb = 512 * g
                    jmax = jstop if jstop is not None else 4 * g + 3
                    jA, jB = 2 * p, 2 * p + 1
                    aA, aB = max(128 * jA, gb), max(128 * jB, gb)
                    stp = pst.tile([128, 1024], F32, tag="st", name=f"stp_{g}_{p}")
                    ptt = ptp.tile([128, 1024], FP16, tag="pt", name=f"ptt_{g}_{p}")
                    for jj, a, col in ((jA, aA, 0), (jB, aB, 512)):
                        kt = kTg[jj // 4][:, ts(jj % 4, 128)]
                        for m in range(a // 512, g + 1):
                            lo, hi = max(a, 512 * m), min(gb + 512, 512 * m + 512)
                            nc.tensor.matmul(
                                stp[:, col + lo - gb:col + hi - gb],
                                kt, qTg[m][:, lo - 512 * m:hi - 512 * m],
                                start=True, stop=True,
                            )
                    if jB >= 4 * g:
                        for jj, a, col in ((jA, aA, 0), (jB, aB, 512)):
                            nc.scalar.activation(
                                ptt[:, col + a - gb:col + 512],
                                stp[:, col + a - gb:col + 512], EXP, scale=0.125,
                            )
                    else:
                        nc.scalar.activation(ptt[:], stp[:], EXP, scale=0.125)
                    for jj, a, col in ((jA, aA, 0), (jB, aB, 512)):
                        if jj >= 4 * g:
                            nc.vector.tensor_mul(
                                ptt[:, col + a - gb:col + a - gb + 128],
                                ptt[:, col + a - gb:col + a - gb + 128], msk_sb,
                            )
                        nc.tensor.matmul(
                            acc[:, a - gb:512], v1g[jj // 4][:, jj % 4, 0:65],
                            ptt[:, col + a - gb:col + 512],
                            start=(jj == 0), stop=(jj == jmax),
                        )

                for g in (0, 2, 3, 1):  # big groups in the middle: their
                    # epilogues overlap compute and the tail ends on a short one
                    acc = pso.tile([65, 512], F32, tag="o", name=f"outT_acc{g}")
                    if g == 1:
                        # final group: put the diag pairs (exp+mask path) in the
                        # middle so the tail-critical last PV has no exp wait
                        pair_order, jstop = (0, 2, 3, 1), 3
                    else:
                        pair_order, jstop = range(2 * g + 2), None
                    for p in pair_order:
                        do_pair(g, p, acc, jstop)
                    ob = osb.tile([65, 512], F32, tag="ob", name=f"ob{g}")
                    nq = 4 if g == 1 else 2
                    for h in range(nq):
                        lo, hi = 512 * h // nq, 512 * (h + 1) // nq
                        nc.vector.tensor_copy(ob[:, lo:hi], acc[:, lo:hi])
                        nc.sync.dma_start(out=out[g, :, lo:hi], in_=ob[:, lo:hi])
    nc.compile()
    return nc


_NC_CACHE = None


def _get_nc() -> bass.Bass:
    global _NC_CACHE
    if _NC_CACHE is None:
        _NC_CACHE = build_nc()
    return _NC_CACHE


def make_in_maps(x: np.ndarray, W: np.ndarray, b: np.ndarray) -> list[dict]:
    cst = np.zeros((128, CST_W), dtype=np.float16)
    # w chunks: cst[p, j*192+m] = W[j*128+p, m]
    cst[:, :NJ * 3 * HD] = (
        W.astype(np.float16).reshape(NJ, 128, 3 * HD).transpose(1, 0, 2).reshape(128, NJ * 3 * HD)
    )
    cst[:, 1536] = b[0:128].astype(np.float16)
    cst[0:64, 1537] = b[128:192].astype(np.float16)
    cst[:, 1538:1666] = np.triu(np.ones((128, 128), dtype=np.float16))  # keep s <= t
    cst = np.ascontiguousarray(cst)
    in_maps = []
    for core in range(N_CORES):
        xtc = np.ascontiguousarray(x[core].astype(np.float16).T)
        in_maps.append({"xt": xtc, "cst": cst})
    return in_maps


def run(x, W, b, trace: bool = False):
    """Returns (output [B, T, HD] fp32, BassKernelResults)."""
    x, W, b = np.asarray(x), np.asarray(W), np.asarray(b)
    nc = _get_nc()
    res = run_bass_kernel_spmd(nc, make_in_maps(x, W, b), list(range(N_CORES)), trace=trace)
    bv = b[128:192].astype(np.float32)  # v bias is separable: added post-softmax
    outs = []
    for i in range(N_CORES):
        o = res.results[i]["out"]  # [NG, 65, 512] unnormalized, transposed, no v-bias
        y = (o[:, 0:HD, :] / o[:, HD:HD + 1, :]).transpose(0, 2, 1).reshape(T, HD) + bv
        outs.append(y)
    return np.stack(outs, axis=0).astype(np.float32), res


def kernel(x, W, b) -> np.ndarray:
    out, _ = run(x, W, b)
    return out


# revision 64
# speedup vs baseline: 1.0121x; 1.0121x over previous
"""Single-head causal attention (B=8, T=2048, C=1024, head_dim=64) on 8 TRN2 NeuronCores.

Sharding: data-parallel over batch -- one batch element per core, qkv weights
replicated. Host prep per core: x[b] transposed to [C, T] fp16; W pre-packed into
the SBUF chunk layout.

v7 schedule. Tile dependencies are tile-granular, so everything the attention
phase consumes is split into PER-GROUP tiles (kTg/qTg/vTg/v1g per 512-column
group) and ST matmuls are split at group boundaries; attention group g then
only waits on group <= g evacuations, not the whole projection epilogue.
  - projection: kqT/vT accumulate j-outer in 8 PSUM banks, paced by chunked
    x DMAs (first chunk split in four so the first matmul starts early).
  - boundary: K/Q/V evacuated per group with fused bias+cast (Q straight
    from PSUM rows 64:128 to SBUF rows 0:64 -- engines can shift partition
    base); Vector does groups 0-1, Scalar groups 2-3; v1 tiles via
    DMA-transpose XBAR on sync. PE warmups (targeting the first-freed PSUM
    bank) bridge the pipeline-fill latency to hold the 2.4GHz p-state.
  - attention (t-group outer): ST pieces -> exp on Scalar -> diag masks on
    Vector -> PV accumulate [65, 512] (denominator in row 64 via the ones
    column of v1). Output leaves UNNORMALIZED as [4, 65, 512] f32 straight
    from an SBUF copy; host divides + transposes.
"""

import numpy as np

import concourse.bass as bass
import concourse.mybir as mybir
from concourse import bacc
from concourse.bass import ts
from concourse.bass_utils import run_bass_kernel_spmd
from concourse.tile import TileContext

B, T, C = 8, 2048, 1024
HD = 64
N_CORES = 8
NJ = C // 128  # contraction chunks for the qkv projection
NT = T // 128  # 128-row tiles along T
NG = T // 512  # 512-col groups along T
FP16 = mybir.dt.float16
IDENT = mybir.ActivationFunctionType.Identity
CST_W = 8 * 192 + 2 + 128 + 64 + 128 + 128  # 1986 (host layout kept from v1)
CB = 192  # cstA holds cols [0:192) = W chunk 0; cstB the rest
F32 = mybir.dt.float32
EXP = mybir.ActivationFunctionType.Exp


def build_nc() -> bass.Bass:
    nc = bacc.Bacc(None, target_bir_lowering=False)
    # w is pre-packed on host: [128, NJ*192] with w[p, j*192+m] = W[j*128+p, m]
    xt = nc.declare_dram_parameter("xt", [C, T], FP16, isOutput=False)
    cst = nc.declare_dram_parameter("cst", [128, CST_W], FP16, isOutput=False)
    # unnormalized output: per 512-col group, rows 0:64 = sum(P v), row 64 = sum(P)
    out = nc.declare_dram_parameter("out", [NG, HD + 1, 512], F32, isOutput=True)

    with TileContext(nc) as tc:
        with (
            tc.tile_pool(name="consts", bufs=1) as consts,
            tc.tile_pool(name="xtp", bufs=NJ + 3) as xtp,
            tc.tile_pool(name="kqv", bufs=1) as kqv,
            tc.tile_pool(name="ptp", bufs=4) as ptp,
            tc.tile_pool(name="osb", bufs=2) as osb,
        ):
            # per-chunk W tiles (separate tiles => per-chunk DMA dependencies)
            cstW = [consts.tile([128, CB], FP16, name=f"cstW{j}") for j in range(NJ)]
            cstM = consts.tile([128, CST_W - 1536], FP16)  # biases | msk
            msk_sb = cstM[:, 2:130]
            wu_sb = consts.tile([1, 512], FP16)
            nc.vector.memset(wu_sb[:], 1.0)
            bias32 = consts.tile([128, 2], F32)

            kTg = [kqv.tile([64, 512], FP16, name=f"kTg{n}") for n in range(NG)]
            qTg = [kqv.tile([64, 512], FP16, name=f"qTg{n}") for n in range(NG)]
            # v for group pairs (0,1) and (2,3) stacked on the partition axis
            vTp = [kqv.tile([128, 512], FP16, name=f"vTp{h}") for h in range(2)]
            v1g = [kqv.tile([128, 4, 80], FP16, name=f"v1g{n}") for n in range(NG)]
            for n in range(NG):
                nc.vector.memset(v1g[n][:, :, HD:HD + 1], 1.0)

            # --- DMAs: interleaved so W chunk j and x chunk j both land just
            # before the projection needs them (per-queue FIFO, ~150 GB/s each)
            xts = [xtp.tile([128, T], FP16, tag="xt", name=f"xt{j}") for j in range(NJ)]
            xt0h = [xtp.tile([128, 1024], FP16, tag="xt0", name=f"xt0h{h}") for h in range(2)]
            with tc.high_priority():
                nc.scalar.dma_start(out=cstW[0][:], in_=cst[:, 0:CB])
                nc.sync.dma_start(out=xt0h[0][:], in_=xt[0:128, 0:1024])
            nc.sync.dma_start(out=xt0h[1][:], in_=xt[0:128, 1024:2048])
            nc.scalar.dma_start(out=cstW[1][:], in_=cst[:, CB:2 * CB])
            nc.sync.dma_start(out=xts[1][:], in_=xt[128:256, :])
            nc.scalar.dma_start(out=xts[2][:], in_=xt[256:384, :])
            nc.scalar.dma_start(out=cstW[2][:], in_=cst[:, 2 * CB:3 * CB])
            nc.scalar.dma_start(out=cstW[3][:], in_=cst[:, 3 * CB:4 * CB])
            nc.scalar.dma_start(out=xts[3][:], in_=xt[384:512, :])
            nc.sync.dma_start(out=xts[4][:], in_=xt[512:640, :])
            nc.scalar.dma_start(out=cstW[4][:], in_=cst[:, 4 * CB:5 * CB])
            nc.scalar.dma_start(out=cstW[5][:], in_=cst[:, 5 * CB:6 * CB])
            nc.scalar.dma_start(out=xts[5][:], in_=xt[640:768, :])
            nc.sync.dma_start(out=xts[6][:], in_=xt[768:896, :])
            nc.scalar.dma_start(out=cstW[6][:], in_=cst[:, 6 * CB:7 * CB])
            nc.scalar.dma_start(out=cstW[7][:], in_=cst[:, 7 * CB:8 * CB])
            nc.scalar.dma_start(out=xts[7][:], in_=xt[896:1024, :])
            nc.sync.dma_start(out=cstM[:], in_=cst[:, 1536:])
            nc.vector.tensor_copy(bias32[:, 0:1], cstM[:, 0:1])
            nc.vector.tensor_copy(bias32[0:64, 1:2], cstM[0:64, 1:2])

            with tc.tile_pool(name="psp", bufs=6, space=bass.MemorySpace.PSUM) as psp:
                # PSUM bank choreography (empirical: the attention pst pool's
                # first slot lands on banks 4-5 = psp emission tiles #5,#6).
                # v accumulators for group pairs share banks: group n at
                # partitions 0:64, group n+1 at 64:128. 6 banks total, leaving
                # two banks of slack for the attention pools.
                kq_accs, vsh = [None] * NG, [None] * 2
                kq_accs[1] = psp.tile([128, 512], F32, tag="p", name="kq_acc1")
                kq_accs[2] = psp.tile([128, 512], F32, tag="p", name="kq_acc2")
                vsh[1] = psp.tile([128, 512], F32, tag="p", name="vsh1")
                kq_accs[3] = psp.tile([128, 512], F32, tag="p", name="kq_acc3")
                kq_accs[0] = psp.tile([128, 512], F32, tag="p", name="kq_acc0")
                vsh[0] = psp.tile([128, 512], F32, tag="p", name="vsh0")
                v_accs = [vsh[0][0:64, :], vsh[0][64:128, :],
                          vsh[1][0:64, :], vsh[1][64:128, :]]
                # no lead warmups: the boost budget is a power duty-cycle --
                # every wasted column costs; the ramp clock starts when the
                # first real matmul issues
                for j in range(NJ):
                    first, last = j == 0, j == NJ - 1
                    if last:  # interleave so accumulators stop (and free) early
                        order = [kv for n in range(NG) for kv in (("kq", n), ("v", n))]
                    else:
                        order = [("kq", n) for n in range(NG)] + [("v", n) for n in range(NG)]
                    for kind, n in order:
                        if j == 0:
                            src = xt0h[n // 2][:, ts(n % 2, 512)]
                        else:
                            src = xts[j][:, ts(n, 512)]
                        if kind == "kq":
                            nc.tensor.matmul(
                                kq_accs[n][:], cstW[j][:, 0:128], src,
                                start=first, stop=last,
                            )
                        else:
                            nc.tensor.matmul(
                                v_accs[n], cstW[j][:, 128:192], src,
                                start=first, stop=last, skip_group_check=True,
                            )
                # boundary: evacuate K/Q per group with fused bias + cast (Q
                # straight from PSUM rows 64:128 to SBUF rows 0:64); the V
                # bias is separable through the softmax and added on HOST, so
                # each V group-pair evacuates in ONE fused cast-copy.
                # Vector: groups 0-1 + V01; Scalar: groups 2-3 + V23.
                ev_dve = lambda o, i, b: nc.vector.tensor_scalar_add(o, i, b)
                ev_act = lambda o, i, b: nc.scalar.activation(o, i, IDENT, bias=b)
                # DVE chain: group 0 first, then V pair 01, group 1, group 3
                ev_dve(kTg[0][:], kq_accs[0][0:64, :], bias32[0:64, 0:1])
                ev_dve(qTg[0][:], kq_accs[0][64:128, :], bias32[64:128, 0:1])
                nc.vector.tensor_copy(vTp[0][:], vsh[0][:])
                nc.sync.dma_start(
                    out=v1g[0][:, 0:2, 0:HD], in_=vTp[0][0:64, 0:256], transpose=True)
                nc.sync.dma_start(
                    out=v1g[0][:, 2:4, 0:HD], in_=vTp[0][0:64, 256:512], transpose=True)
                ev_dve(kTg[1][:], kq_accs[1][0:64, :], bias32[0:64, 0:1])
                ev_dve(qTg[1][:], kq_accs[1][64:128, :], bias32[64:128, 0:1])
                # ACT chain
                ev_act(kTg[2][:], kq_accs[2][0:64, :], bias32[0:64, 0:1])
                ev_act(qTg[2][:], kq_accs[2][64:128, :], bias32[64:128, 0:1])
                nc.scalar.copy(vTp[1][:], vsh[1][:])
                nc.sync.dma_start(
                    out=v1g[2][:, :, 0:HD], in_=vTp[1][0:64, :], transpose=True)
                nc.sync.dma_start(
                    out=v1g[3][:, :, 0:HD], in_=vTp[1][64:128, :], transpose=True)
                ev_act(kTg[3][:], kq_accs[3][0:64, :], bias32[0:64, 0:1])
                ev_act(qTg[3][:], kq_accs[3][64:128, :], bias32[64:128, 0:1])
                nc.sync.dma_start(
                    out=v1g[1][:, :, 0:HD], in_=vTp[0][64:128, :], transpose=True)
                # minimal boundary filler: enough to cover the evac/exp/v1
                # pipeline-fill, small enough not to waste the boost window
                for r in range(2):
                    nc.tensor.matmul(kq_accs[0][:], wu_sb[:, 0:128], wu_sb[:], start=True, stop=True)

            # --- attention, t-group outer: ST pieces for two s-chunks share a
            # [128,1024] PSUM tile and one exp (Scalar does ONLY exp); diag
            # masks on Vector; PV accumulates [65, 512], denominator in row 64 ---
            with (
                tc.tile_pool(name="pst", bufs=3, space=bass.MemorySpace.PSUM) as pst,
                tc.tile_pool(name="pso", bufs=2, space=bass.MemorySpace.PSUM) as pso,
            ):
                def do_pair(g, p, acc, jstop=None):
                    gb = 512 * g
                    jmax = jstop if jstop is not None else 4 * g + 3
                    jA, jB = 2 * p, 2 * p + 1
                    aA, aB = max(128 * jA, gb), max(128 * jB, gb)
                    stp = pst.tile([128, 1024], F32, tag="st", name=f"stp_{g}_{p}")
                    ptt = ptp.tile([128, 1024], FP16, tag="pt", name=f"ptt_{g}_{p}")
                    for jj, a, col in ((jA, aA, 0), (jB, aB, 512)):
                        kt = kTg[jj // 4][:, ts(jj % 4, 128)]
                        for m in range(a // 512, g + 1):
                            lo, hi = max(a, 512 * m), min(gb + 512, 512 * m + 512)
                            nc.tensor.matmul(
                                stp[:, col + lo - gb:col + hi - gb],
                                kt, qTg[m][:, lo - 512 * m:hi - 512 * m],
                                start=True, stop=True,
                            )
                    if jB >= 4 * g:
                        for jj, a, col in ((jA, aA, 0), (jB, aB, 512)):
                            nc.scalar.activation(
                                ptt[:, col + a - gb:col + 512],
                                stp[:, col + a - gb:col + 512], EXP, scale=0.125,
                            )
                    else:
                        nc.scalar.activation(ptt[:], stp[:], EXP, scale=0.125)
                    for jj, a, col in ((jA, aA, 0), (jB, aB, 512)):
                        if jj >= 4 * g:
                            nc.vector.tensor_mul(
                                ptt[:, col + a - gb:col + a - gb + 128],
                                ptt[:, col + a - gb:col + a - gb + 128], msk_sb,
                            )
                        nc.tensor.matmul(
                            acc[:, a - gb:512], v1g[jj // 4][:, jj % 4, 0:65],
                            ptt[:, col + a - gb:col + 512],
                            start=(jj == 0), stop=(jj == jmax),
                        )

                for g in (0, 2, 3, 1):  # big groups in the middle: their
                    # epilogues overlap compute and the tail ends on a short one
                    acc = pso.tile([65, 512], F32, tag="o", name=f"outT_acc{g}")
                    if g == 1:
                        # final group: put the diag pairs (exp+mask path) in the
                        # middle so the tail-critical last PV has no exp wait
                        pair_order, jstop = (0, 2, 3, 1), 3
                    else:
                        pair_order, jstop = range(2 * g + 2), None
                    for p in pair_order:
                        do_pair(g, p, acc, jstop)
                    ob = osb.tile([65, 512], F32, tag="ob", name=f"ob{g}")
                    if g == 1:
                        # final group: two independent copy->DMA lanes
                        # (Vector->sync and Scalar->scalar; exps are done)
                        nc.vector.tensor_copy(ob[:, 0:256], acc[:, 0:256])
                        nc.sync.dma_start(out=out[g, :, 0:256], in_=ob[:, 0:256])
                        nc.scalar.copy(ob[:, 256:512], acc[:, 256:512])
                        nc.scalar.dma_start(out=out[g, :, 256:512], in_=ob[:, 256:512])
                    else:
                        for h in range(2):
                            lo, hi = 256 * h, 256 * h + 256
                            nc.vector.tensor_copy(ob[:, lo:hi], acc[:, lo:hi])
                            nc.sync.dma_start(out=out[g, :, lo:hi], in_=ob[:, lo:hi])
    nc.compile()
    return nc


_NC_CACHE = None


def _get_nc() -> bass.Bass:
    global _NC_CACHE
    if _NC_CACHE is None:
        _NC_CACHE = build_nc()
    return _NC_CACHE


def make_in_maps(x: np.ndarray, W: np.ndarray, b: np.ndarray) -> list[dict]:
    cst = np.zeros((128, CST_W), dtype=np.float16)
    # w chunks: cst[p, j*192+m] = W[j*128+p, m]
    cst[:, :NJ * 3 * HD] = (
        W.astype(np.float16).reshape(NJ, 128, 3 * HD).transpose(1, 0, 2).reshape(128, NJ * 3 * HD)
    )
    cst[:, 1536] = b[0:128].astype(np.float16)
    cst[0:64, 1537] = b[128:192].astype(np.float16)
    cst[:, 1538:1666] = np.triu(np.ones((128, 128), dtype=np.float16))  # keep s <= t
    cst = np.ascontiguousarray(cst)
    in_maps = []
    for core in range(N_CORES):
        xtc = np.ascontiguousarray(x[core].astype(np.float16).T)
        in_maps.append({"xt": xtc, "cst": cst})
    return in_maps


def run(x, W, b, trace: bool = False):
    """Returns (output [B, T, HD] fp32, BassKernelResults)."""
    x, W, b = np.asarray(x), np.asarray(W), np.asarray(b)
    nc = _get_nc()
    res = run_bass_kernel_spmd(nc, make_in_maps(x, W, b), list(range(N_CORES)), trace=trace)
    bv = b[128:192].astype(np.float32)  # v bias is separable: added post-softmax
    outs = []
    for i in range(N_CORES):
        o = res.results[i]["out"]  # [NG, 65, 512] unnormalized, transposed, no v-bias
        y = (o[:, 0:HD, :] / o[:, HD:HD + 1, :]).transpose(0, 2, 1).reshape(T, HD) + bv
        outs.append(y)
    return np.stack(outs, axis=0).astype(np.float32), res


def kernel(x, W, b) -> np.ndarray:
    out, _ = run(x, W, b)
    return out
